# revision 2
# baseline (speedup 1.0000x reference)
"""Triangular pairwise channel product on 8 Trainium2 NeuronCores.

out[b,h,w,k] = x[b,h,w,i_k] * x[b,h,w,j_k]  for the C*(C-1)/2 pairs
(i<j) in row-major (np.triu_indices) order.

Sharding: pure data parallel over batch — core c takes x[2c:2c+2].
Per core the 2*64*64 = 8192 spatial positions map to 128 SBUF
partitions (b_loc*64+h) x 64 groups (w).  For each group-block of G
positions, block i of the output (pairs (i, i+1..63)) is one fp32
tensor_tensor multiply whose first operand is x[:, :, i] broadcast via
a step-0 access pattern.

The kernel is HBM-store bound: 66 MB of output per core vs ~420 GB/s
of DMA ≈ 157 us of unavoidable store time.  fp32 tensor_tensor runs at
1 elem/cycle/lane on DVE (~123 G elem/s) which alone cannot keep the
store stream fed (measured 175 us DVE-busy vs 165 us DMA-busy at
baseline, DMA idle 22%).  Fix: split each group-block's 63 multiplies
between DVE (large blocks i < I0) and GPSIMD (small tail blocks,
~2.6 cyc/elem but idle otherwise; separate SBUF ports, fp32 TT on DVE
is 1x-mode so the shared port pair is never contended; all DMAs are
HWDGE so the Q7 cores have no descriptor work).  Iteration sizes taper
up at the start (stores begin early) and down at the end (small final
drain); stores alternate between the SP and ACT HWDGE rings so
per-transfer fixed latencies overlap.
"""

import numpy as np

import concourse.bacc as bacc
import concourse.bass as bass
import concourse.mybir as mybir
import concourse.tile as tile
from concourse.bass_utils import run_bass_kernel_spmd

B, H, W, C = 16, 64, 64, 64
K = C * (C - 1) // 2  # 2016
N_CORES = 8
BP = B // N_CORES  # batch rows per core
P = BP * H         # 128 SBUF partitions
G_TOTAL = W        # position groups per partition
# Ascending head so the first store launches early, steady G=8 middle
# (compute < store per group-block), descending tail so the post-compute
# drain stays small.
G_ITERS = [2, 6, 8, 8, 8, 8, 8, 8, 5, 3]
assert sum(G_ITERS) == W
G0 = G_ITERS[0]
# Blocks i >= I0 (widths 63-I0 .. 1) run on GPSIMD, the rest on DVE.
I0 = 36
FP = mybir.dt.float32

_row = [0]
for _i in range(C):
    _row.append(_row[-1] + C - 1 - _i)

_nc_cache = None


def build_bass() -> bass.Bass:
    # Bacc (not plain Bass): its compile() pipeline runs
    # generate_event_semaphores, which splits multi-wait instructions to
    # satisfy the TRN2 1-wait-per-instruction codegen limit.
    nc = bacc.Bacc(
        "TRN2",
        target_bir_lowering=False,
        debug=False,
        num_devices=N_CORES,
    )
    x = nc.dram_tensor("x", [P, G_TOTAL, C], FP, kind="ExternalInput")
    y = nc.dram_tensor("y", [P, G_TOTAL, K], FP, kind="ExternalOutput")

    with tile.TileContext(nc) as tc:
        with (
            tc.tile_pool(name="xin", bufs=1) as xpool,
            tc.tile_pool(name="out", bufs=2) as opool,
        ):
            # Preload the input in two pieces: iteration 0's chunk on the
            # SP ring (out0 isn't due on it immediately), the rest on the
            # ACT ring so neither load queues behind output stores.
            xt0 = xpool.tile([P, G0, C], FP, tag="x0")
            nc.sync.dma_start(out=xt0[:], in_=x[:, 0:G0, :])
            xtr = xpool.tile([P, G_TOTAL - G0, C], FP, tag="xr")
            nc.scalar.dma_start(out=xtr[:], in_=x[:, G0:, :])

            g_off = 0
            for it, Gi in enumerate(G_ITERS):
                if it == 0:
                    xg = xt0[:, :, :]
                else:
                    xg = xtr[:, g_off - G0 : g_off - G0 + Gi, :]

                ot = opool.tile([P, Gi, K], FP, tag="ot")
                for i in range(C - 1):
                    w = C - 1 - i
                    a = xg[:, :, i : i + 1].broadcast_to([P, Gi, w])
                    b = xg[:, :, i + 1 : C]
                    eng = nc.vector if i < I0 else nc.gpsimd
                    eng.tensor_mul(ot[:, :, _row[i] : _row[i] + w], a, b)

                # Full 2016-channel rows -> contiguous per-partition DRAM
                # runs; alternate HWDGE rings so fixed costs overlap.
                ring = nc.sync if it % 2 == 0 else nc.scalar
                ring.dma_start(out=y[:, g_off : g_off + Gi, :], in_=ot[:])
                g_off += Gi

    nc.finalize()
    return nc


def make_in_maps(x: np.ndarray) -> list[dict[str, np.ndarray]]:
    x = np.ascontiguousarray(x, dtype=np.float32)
    return [
        {"x": x[c * BP : (c + 1) * BP].reshape(P, G_TOTAL, C)} for c in range(N_CORES)
    ]


def kernel(**inputs: np.ndarray) -> np.ndarray:
    global _nc_cache
    if _nc_cache is None:
        _nc_cache = build_bass()
    res = run_bass_kernel_spmd(
        _nc_cache, make_in_maps(inputs["inputs"]), list(range(N_CORES))
    ).results
    return np.concatenate(
        [res[c]["y"].reshape(BP, H, W, K) for c in range(N_CORES)], axis=0
    )
